# revision 4
# baseline (speedup 1.0000x reference)
"""AttnBlock (GroupNorm -> QKV 1x1 conv -> attention -> proj -> residual) on 8 trn2 cores.

Sharding: data-parallel over batch (32 batches -> 4 per core), weights replicated.

FP8 version: all five matmul groups (QK, V, scores, O=V.T@E, proj) run as
fp8e4(e4m3, TRN flavor: max 240) x fp8e4 with MatmulPerfMode.DoubleRow —
each MM covers a K=256 contraction (pairs of 128-chunks on dim1 of the
tile, 3D APs [128, 2, free]) at the same 512-cycle stream, ~2x FLOP rate.

Scale management (so nothing exceeds the 240 -> Inf conversion cliff):
- hn quantized to fp8 at scale 1 (|hn| ~< 6).
- weights shipped as fp8(16*w) (sigma ~0.7); biases bq,bk,bv shipped as 16*b.
- q' = 16q, k' = 16k in fp8 (|q'| ~< 90)  -> scores acc = 256*(q.k)
- E = exp(acc * C^-0.5/256 - 2) in fp8 (max ~123; softmax shift-invariant,
  normalization deferred so the constant e^-2 cancels in rowsum)
- V' = 16v fp8; O_acc = V'.T @ E = 16*(sum E v); OT = O_acc * 2^-6 fp8 (max ~95)
- P_acc = OT.T@(16wp) = 4*(sum E v)@wp; final = P_acc * 1/(4*rowsum) + (x+bp):
  the ones-column for the rowsum matmul is memset to 4.0 so the reciprocal
  directly yields 1/(4S).
Per-element fp8 rel err ~2%; attention branch is only ~9% of output norm,
so final rel err ~7e-3 (simulated) vs the 2e-2 gate.

Layout strategy per core (T=1024 tokens, C=512 channels per batch, B=4):
- x DMA'd twice: bf16 channel-major x^T tiles via DMA-transpose (feeds
  GroupNorm stats + apply), f32 token-major (residual).
- GroupNorm stats per channel via bn_stats over tokens, tiny group-combine
  matmuls, fused tensor_scalar apply producing fp8 hn^T.
- scores^T[tk,tq] = K^T.T @ Q^T per (tk tile, tq 512-chunk); exp on ScalarE
  (scale+bias folded), fp8 E^T; row sums via DVE partial accumulation +
  one float32r ones-column matmul; normalization deferred past the output
  projection and fused into the final residual pass (scalar_tensor_tensor).
"""

import os
import sys

sys.path.insert(0, "/opt/trn_rl_repo")

import numpy as np
import ml_dtypes

import concourse.bass as bass
import concourse.mybir as mybir
import concourse.tile as tile
from concourse import bacc
from concourse.bass_utils import run_bass_kernel_spmd

BF16 = mybir.dt.bfloat16
FP8 = mybir.dt.float8e4
F32 = mybir.dt.float32
AF = mybir.ActivationFunctionType
ALU = mybir.AluOpType
DR = mybir.MatmulPerfMode.DoubleRow

NCORES = 8
B = 4          # batches per core
T = 1024       # tokens (h*w) per batch
C = 512        # channels
G = 32         # groups
GS = C // G    # 16 channels per group
NC4 = C // 128   # 4 channel chunks
NT8 = T // 128   # 8 token tiles
EPS = 1e-6
SCALE = C ** -0.5
OFF = 2.0        # exp offset: E = exp(s - OFF), cancels via deferred norm
WS = 16.0        # weight/bias prescale (host side)
OSH = 0.015625   # 2^-6 OT writeout scale
RS = 4.0         # rowsum ones value: 256*OSH so rcols = 1/(4S) matches P_acc scale


def build_kernel(repeat=1, bench=False):
    nc = bacc.Bacc("TRN2", target_bir_lowering=False, debug=False)

    if bench:
        # timing-only variant: no external I/O beyond a tiny debug output, so
        # the axon tunnel ships ~nothing per call; x lives in internal DRAM
        # (zero-filled below), weights/biases are memset on SBUF directly.
        xt_bf = nc.dram_tensor("xt_bf_i", [B, T, C], BF16, kind="Internal")
        x_f = nc.dram_tensor("x_f_i", [B, T, C], F32, kind="Internal")
        out_d = nc.dram_tensor("out_i", [B, T, C], F32, kind="Internal")
        out_dbg = nc.dram_tensor("out_dbg", [1, T], F32, kind="ExternalOutput")
    else:
        xt_bf = nc.dram_tensor("xt_bf", [B, T, C], BF16, kind="ExternalInput")
        x_f = nc.dram_tensor("x_f", [B, T, C], F32, kind="ExternalInput")
        out_d = nc.dram_tensor("out", [B, T, C], F32, kind="ExternalOutput")
    if not bench:
        w_d = {}
        for w in ("wq", "wk", "wv", "wp"):
            w_d[w] = nc.dram_tensor(w, [C, C], FP8, kind="ExternalInput")
        b_d = {}
        for b in ("bq", "bk", "gns", "gnb"):
            b_d[b] = nc.dram_tensor(b, [C], F32, kind="ExternalInput")
        for b in ("bv", "bp"):
            b_d[b] = nc.dram_tensor(b, [C], F32, kind="ExternalInput")
        gsel_d = nc.dram_tensor("gsel", [C, G], F32, kind="ExternalInput")
        gselT_d = nc.dram_tensor("gselT", [G, C], F32, kind="ExternalInput")

    with tile.TileContext(nc) as tc:
        with tc.tile_pool(name="const", bufs=1) as const, \
             tc.tile_pool(name="work", bufs=1) as work, \
             tc.tile_pool(name="psum", bufs=6, space="PSUM") as psum, \
             tc.tile_pool(name="dscratch", bufs=2, space="DRAM") as dscratch:

            # ---- constants ----
            w_t = {}
            for w in ("wq", "wk", "wv", "wp"):
                wt = const.tile([128, NC4, C], FP8, name=f"{w}_t")
                if bench:
                    nc.vector.memset(wt, 0.0)
                else:
                    nc.sync.dma_start(out=wt, in_=w_d[w].ap().rearrange("(i p) c -> p i c", p=128))
                w_t[w] = wt
            b_c = {}
            for b in ("bq", "bk", "gns", "gnb"):
                bc = const.tile([128, NC4], F32, name=f"{b}_c")
                if bench:
                    nc.vector.memset(bc, 0.0)
                else:
                    nc.sync.dma_start(out=bc, in_=b_d[b].ap().rearrange("(i p) -> p i", p=128))
                b_c[b] = bc
            def row_bcast(dram_t):
                src = dram_t.ap()
                return bass.AP(tensor=src.tensor, offset=src.offset, ap=[[0, 128]] + list(src.ap))

            bv_b = const.tile([128, C], F32)
            bp_b = const.tile([128, C], F32)
            gsel_t = const.tile([128, NC4, G], F32)
            gselT_t = const.tile([G, C], F32)
            if bench:
                nc.vector.memset(bv_b, 0.0)
                nc.vector.memset(bp_b, 0.0)
                nc.vector.memset(gsel_t, 1.0 / GS)
                nc.vector.memset(gselT_t, 1.0)
            else:
                nc.sync.dma_start(out=bv_b, in_=row_bcast(b_d["bv"]))
                nc.sync.dma_start(out=bp_b, in_=row_bcast(b_d["bp"]))
                nc.sync.dma_start(out=gsel_t, in_=gsel_d.ap().rearrange("(i p) g -> p i g", p=128))
                nc.sync.dma_start(out=gselT_t, in_=gselT_d.ap())
            ones_f = const.tile([128, 1], F32)
            nc.vector.memset(ones_f, RS)
            ones_r = const.tile([128, 1], mybir.dt.float32r)
            nc.vector.tensor_copy(out=ones_r, in_=ones_f)
            eps32 = const.tile([G, 1], F32)
            nc.vector.memset(eps32, EPS)
            noff = const.tile([128, 1], F32)
            nc.vector.memset(noff, -OFF)
            # prime the ScalarE exp/ln table set while the weight DMAs run, so
            # the first GroupNorm rstd doesn't eat the ~2.7us table load
            warm = const.tile([1, 1], F32)
            nc.scalar.activation(out=warm, in_=eps32[0:1, 0:1], func=AF.Exp, scale=1.0)

            if bench:
                # zero-fill internal x so the repeated body is NaN-free
                zf = work.tile([128, NT8, C], F32, tag="xf", bufs=2)
                nc.vector.memset(zf, 0.0)
                zb = work.tile([128, NT8, C], BF16, tag="zb")
                nc.vector.memset(zb, 0.0)
                for ib in range(B):
                    nc.sync.dma_start(out=x_f.ap()[ib].rearrange("(i p) c -> p i c", p=128), in_=zf)
                    nc.sync.dma_start(out=xt_bf.ap()[ib].rearrange("(i p) c -> p i c", p=128), in_=zb)

            for _rep in range(repeat):
              # ---- prologue: GroupNorm for all batches (keeps the steady-state
              # PE stream free of GN work at batch boundaries) ----
              hnT_l = []
              for ib in range(B):
                  xT = work.tile([128, NC4, T], BF16, tag="xT", bufs=2, name=f"xT{ib}")
                  for ci in range(NC4):
                      nc.sync.dma_start(
                          out=xT[:, ci, :],
                          in_=xt_bf.ap()[ib, :, ci * 128:(ci + 1) * 128],
                          transpose=True,
                      )
                  bn6 = work.tile([128, NC4, 2, 6], F32, tag="bn6", bufs=2, name=f"bn6_{ib}")
                  mv = work.tile([128, NC4, 2], F32, tag="mv", bufs=2, name=f"mv{ib}")
                  st2 = work.tile([128, NC4, 2], F32, tag="st2", bufs=2, name=f"st2_{ib}")
                  for ci in range(NC4):
                      nc.vector.bn_stats(out=bn6[:, ci, 0, :], in_=xT[:, ci, 0:512])
                      nc.vector.bn_stats(out=bn6[:, ci, 1, :], in_=xT[:, ci, 512:1024])
                      nc.vector.bn_aggr(out=mv[:, ci, :], in_=bn6[:, ci, :, :])
                      # st2 = [mean_c, var_c + mean_c^2]
                      nc.vector.tensor_tensor(out=st2[:, ci, 1:2], in0=mv[:, ci, 0:1], in1=mv[:, ci, 0:1], op=ALU.mult)
                      nc.vector.tensor_tensor(out=st2[:, ci, 1:2], in0=st2[:, ci, 1:2], in1=mv[:, ci, 1:2], op=ALU.add)
                      nc.vector.tensor_copy(out=st2[:, ci, 0:1], in_=mv[:, ci, 0:1])
                  gst = psum.tile([G, 2], F32, tag="small", bufs=2, name=f"gst{ib}")
                  for ci in range(NC4):
                      nc.tensor.matmul(gst, gsel_t[:, ci, :], st2[:, ci, :],
                                       start=(ci == 0), stop=(ci == NC4 - 1))
                  # gq cols: 0=mean_g 1=Ex2_g 2=mean^2 3=var 4=ln(var+eps) 5=rstd 6=mean
                  gq = work.tile([G, 7], F32, tag="gq", bufs=2, name=f"gq{ib}")
                  nc.vector.tensor_copy(out=gq[:, 0:2], in_=gst)
                  nc.vector.tensor_tensor(out=gq[:, 2:3], in0=gq[:, 0:1], in1=gq[:, 0:1], op=ALU.mult)
                  nc.vector.tensor_tensor(out=gq[:, 3:4], in0=gq[:, 1:2], in1=gq[:, 2:3], op=ALU.subtract)
                  nc.scalar.activation(out=gq[:, 4:5], in_=gq[:, 3:4], func=AF.Ln, bias=eps32, scale=1.0)
                  nc.scalar.activation(out=gq[:, 5:6], in_=gq[:, 4:5], func=AF.Exp, scale=-0.5)
                  nc.vector.tensor_copy(out=gq[:, 6:7], in_=gq[:, 0:1])
                  # expand to channels + per-channel affine [r', m']
                  rm = work.tile([128, NC4, 2], F32, tag="rm", bufs=2, name=f"rm{ib}")
                  for ci in range(NC4):
                      chq = psum.tile([128, 2], F32, tag="small", bufs=2, name=f"chq{ib}_{ci}")
                      nc.tensor.matmul(chq, gselT_t[:, ci * 128:(ci + 1) * 128], gq[:, 5:7],
                                       start=True, stop=True)
                      # r' = rstd * gns ; m' = gnb - mean * r'
                      nc.vector.tensor_tensor(out=rm[:, ci, 0:1], in0=chq[:, 0:1], in1=b_c["gns"][:, ci:ci + 1], op=ALU.mult)
                      nc.vector.tensor_tensor(out=rm[:, ci, 1:2], in0=chq[:, 1:2], in1=rm[:, ci, 0:1], op=ALU.mult)
                      nc.vector.tensor_tensor(out=rm[:, ci, 1:2], in0=b_c["gnb"][:, ci:ci + 1], in1=rm[:, ci, 1:2], op=ALU.subtract)
                  hnT = work.tile([128, NC4, T], FP8, tag="hnT", bufs=B, name=f"hnT{ib}")
                  for ci in range(NC4):
                      nc.vector.tensor_scalar(
                          out=hnT[:, ci, :], in0=xT[:, ci, :],
                          scalar1=rm[:, ci, 0:1], scalar2=rm[:, ci, 1:2],
                          op0=ALU.mult, op1=ALU.add)
                  hnT_l.append(hnT)

              for ib in range(B):
                  hnT = hnT_l[ib]
                  xf = work.tile([128, NT8, C], F32, tag="xf", bufs=2, name=f"xf{ib}")
                  nc.sync.dma_start(out=xf, in_=x_f.ap()[ib].rearrange("(i p) c -> p i c", p=128))

                  # ---- QKV (DoubleRow: 2 accumulation steps over K=512) ----
                  qT = work.tile([128, NC4, T], FP8, tag="qT", bufs=1, name=f"qT{ib}")
                  kT = work.tile([128, NC4, T], FP8, tag="kT", bufs=1, name=f"kT{ib}")
                  for (wname, bname, dst) in (("wq", "bq", qT), ("wk", "bk", kT)):
                      for co in range(NC4):
                          for h in range(2):
                              acc = psum.tile([128, 512], F32, tag="mm", name=f"acc_{wname}{ib}_{co}_{h}")
                              for kp in range(2):
                                  nc.tensor.matmul(
                                      acc, w_t[wname][:, 2 * kp:2 * kp + 2, co * 128:(co + 1) * 128],
                                      hnT[:, 2 * kp:2 * kp + 2, h * 512:(h + 1) * 512],
                                      start=(kp == 0), stop=(kp == 1), perf_mode=DR)
                              nc.vector.tensor_scalar(
                                  out=dst[:, co, h * 512:(h + 1) * 512], in0=acc,
                                  scalar1=b_c[bname][:, co:co + 1], scalar2=None, op0=ALU.add)
                  V = work.tile([128, NT8, C], FP8, tag="V", bufs=1, name=f"V{ib}")
                  for it in range(NT8):
                      acc = psum.tile([128, 512], F32, tag="mm", name=f"acc_v{ib}_{it}")
                      for kp in range(2):
                          nc.tensor.matmul(acc, hnT[:, 2 * kp:2 * kp + 2, it * 128:(it + 1) * 128],
                                           w_t["wv"][:, 2 * kp:2 * kp + 2, :],
                                           start=(kp == 0), stop=(kp == 1), perf_mode=DR)
                      nc.vector.tensor_tensor(out=V[:, it, :], in0=acc, in1=bv_b, op=ALU.add)

                  # ---- scores^T -> exp -> E, and row sums ----
                  E = work.tile([128, NT8, T], FP8, tag="E", bufs=1, name=f"E{ib}")
                  srow = work.tile([1, T], F32, tag="srow", bufs=2, name=f"srow{ib}")
                  ssb_l = []
                  for h in range(2):
                      ssb = work.tile([128, 512], mybir.dt.float32r, tag="ssb", bufs=2, name=f"ssb{ib}_{h}")
                      for tk in range(NT8):
                          acc = psum.tile([128, 512], F32, tag="mm", name=f"acc_s{ib}_{h}_{tk}")
                          for kp in range(2):
                              nc.tensor.matmul(acc, kT[:, 2 * kp:2 * kp + 2, tk * 128:(tk + 1) * 128],
                                               qT[:, 2 * kp:2 * kp + 2, h * 512:(h + 1) * 512],
                                               start=(kp == 0), stop=(kp == 1), perf_mode=DR)
                          nc.scalar.activation(out=E[:, tk, h * 512:(h + 1) * 512], in_=acc,
                                               func=AF.Exp, scale=SCALE / (WS * WS), bias=noff)
                          if tk == 0:
                              nc.vector.tensor_copy(out=ssb, in_=E[:, tk, h * 512:(h + 1) * 512])
                          else:
                              nc.vector.tensor_tensor(out=ssb, in0=ssb, in1=E[:, tk, h * 512:(h + 1) * 512], op=ALU.add)
                      ssb_l.append(ssb)

                  # ---- O^T = V.T @ E^T (DoubleRow over token pairs) ----
                  # row sums are emitted after the FIRST O group: by then the serial
                  # DVE partial-sum chain has drained (no PE wait), and the rcols
                  # DRAM bounce completes mid-O-phase, well before the projection
                  OT = work.tile([128, NC4, T], FP8, tag="OT", bufs=1, name=f"OT{ib}")
                  for co in range(NC4):
                      for h in range(2):
                          acc = psum.tile([128, 512], F32, tag="mm", name=f"acc_o{ib}_{co}_{h}")
                          for tp in range(4):
                              nc.tensor.matmul(acc, V[:, 2 * tp:2 * tp + 2, co * 128:(co + 1) * 128],
                                               E[:, 2 * tp:2 * tp + 2, h * 512:(h + 1) * 512],
                                               start=(tp == 0), stop=(tp == 3), perf_mode=DR)
                          nc.scalar.activation(out=OT[:, co, h * 512:(h + 1) * 512], in_=acc,
                                               func=AF.Copy, scale=OSH)
                          if co == 0 and h == 0:
                              for hs in range(2):
                                  sums = psum.tile([1, 512], F32, tag="small", bufs=2, name=f"sums{ib}_{hs}")
                                  # float32r runs at full PE rate for N>=256 (plain fp32 is 4x slower)
                                  nc.tensor.matmul(sums, ones_r, ssb_l[hs], start=True, stop=True)
                                  nc.vector.reciprocal(out=srow[:, hs * 512:(hs + 1) * 512], in_=sums)
                              # bounce recip row -> per-token columns
                              dsc = dscratch.tile([T], F32, tag="dsc", name=f"dsc{ib}")
                              nc.sync.dma_start(out=dsc, in_=srow[0:1, :])
                              rcols = work.tile([128, NT8], F32, tag="rcols", bufs=2, name=f"rcols{ib}")
                              nc.sync.dma_start(out=rcols, in_=dsc.rearrange("(j p) -> p j", p=128))

                  # ---- proj + normalize + residual ----
                  xpb = work.tile([128, NT8, C], F32, tag="xpb", bufs=1, name=f"xpb{ib}")
                  for it in range(NT8):
                      nc.gpsimd.tensor_tensor(out=xpb[:, it, :], in0=xf[:, it, :], in1=bp_b, op=ALU.add)
                  fin = work.tile([128, NT8, C], F32, tag="fin", bufs=1, name=f"fin{ib}")
                  for it in range(NT8):
                      acc = psum.tile([128, 512], F32, tag="mm", name=f"acc_p{ib}_{it}")
                      for kp in range(2):
                          nc.tensor.matmul(acc, OT[:, 2 * kp:2 * kp + 2, it * 128:(it + 1) * 128],
                                           w_t["wp"][:, 2 * kp:2 * kp + 2, :],
                                           start=(kp == 0), stop=(kp == 1), perf_mode=DR)
                      nc.vector.scalar_tensor_tensor(out=fin[:, it, :], in0=acc,
                                                     scalar=rcols[:, it:it + 1], in1=xpb[:, it, :],
                                                     op0=ALU.mult, op1=ALU.add)
                  nc.sync.dma_start(out=out_d.ap()[ib].rearrange("(i p) c -> p i c", p=128), in_=fin)
            if bench:
                nc.sync.dma_start(out=out_dbg.ap(), in_=srow)

    nc.compile()
    return nc


def make_selectors():
    cc = np.arange(C)
    gg = np.arange(G)
    sel = (cc[:, None] // GS == gg[None, :]).astype(np.float32)
    gsel = sel / GS            # [C, G] averaging
    gselT = sel.T.copy()       # [G, C] expand
    return gsel, gselT


_NC_CACHE = {}


def _get_nc(repeat=1, bench=False):
    key = (repeat, bench)
    if key not in _NC_CACHE:
        _NC_CACHE[key] = build_kernel(repeat, bench)
    return _NC_CACHE[key]


def make_in_maps(x, norm_scale, norm_bias, wq, bq, wk, bk, wv, bv, wp, bp):
    x = np.asarray(x, dtype=np.float32)
    b, h, w, c = x.shape
    assert (b, h * w, c) == (B * NCORES, T, C)
    xr = np.ascontiguousarray(x.reshape(b, h * w, c))
    xr_bf = xr.astype(ml_dtypes.bfloat16)
    gsel, gselT = make_selectors()
    common = {
        "wq": (WS * np.asarray(wq, np.float32)).astype(ml_dtypes.float8_e4m3),
        "wk": (WS * np.asarray(wk, np.float32)).astype(ml_dtypes.float8_e4m3),
        "wv": (WS * np.asarray(wv, np.float32)).astype(ml_dtypes.float8_e4m3),
        "wp": (WS * np.asarray(wp, np.float32)).astype(ml_dtypes.float8_e4m3),
        "bq": WS * np.asarray(bq, np.float32), "bk": WS * np.asarray(bk, np.float32),
        "bv": WS * np.asarray(bv, np.float32), "bp": np.asarray(bp, np.float32),
        "gns": np.asarray(norm_scale, np.float32), "gnb": np.asarray(norm_bias, np.float32),
        "gsel": gsel, "gselT": gselT,
    }
    in_maps = []
    for i in range(NCORES):
        sl = slice(i * B, (i + 1) * B)
        in_maps.append({"xt_bf": xr_bf[sl], "x_f": xr[sl], **common})
    return in_maps


def run(in_maps, **kw):
    nc = _get_nc()
    try:
        res = run_bass_kernel_spmd(nc, in_maps, core_ids=list(range(NCORES)), **kw)
    except Exception:
        # transient NRT device wedges happen; one retry is usually enough
        import time as _time
        _time.sleep(2.0)
        res = run_bass_kernel_spmd(nc, in_maps, core_ids=list(range(NCORES)), **kw)
    outs = [r["out"] for r in res.results]
    full = np.concatenate(outs, axis=0).reshape(B * NCORES, 32, 32, C)
    return full, res


def kernel(x, norm_scale, norm_bias, wq, bq, wk, bk, wv, bv, wp, bp):
    in_maps = make_in_maps(x, norm_scale, norm_bias, wq, bq, wk, bk, wv, bv, wp, bp)
    full, _ = run(in_maps)
    return full


if __name__ == "__main__":
    rng = np.random.default_rng(0)
    inputs = {
        "x": rng.standard_normal((32, 32, 32, 512), dtype=np.float32),
        "norm_scale": np.ones(512, np.float32),
        "norm_bias": np.zeros(512, np.float32),
    }
    s = 1.0 / np.sqrt(512)
    for nm in ("q", "k", "v", "p"):
        inputs[f"w{nm}"] = rng.standard_normal((512, 512), dtype=np.float32) * s
        inputs[f"b{nm}"] = np.zeros(512, np.float32)
    out = kernel(**inputs)
    print("out", out.shape, out.dtype, float(np.abs(out).max()))


# revision 5
# speedup vs baseline: 5.5989x; 5.5989x over previous
"""AttnBlock (GroupNorm -> QKV 1x1 conv -> attention -> proj -> residual) on 8 trn2 cores.

Sharding: data-parallel over batch (32 batches -> 4 per core), weights replicated.

FP8 version: all five matmul groups (QK, V, scores, O=V.T@E, proj) run as
fp8e4(e4m3, TRN flavor: max 240) x fp8e4 with MatmulPerfMode.DoubleRow —
each MM covers a K=256 contraction (pairs of 128-chunks on dim1 of the
tile, 3D APs [128, 2, free]) at the same 512-cycle stream, ~2x FLOP rate.

Scale management (so nothing exceeds the 240 -> Inf conversion cliff):
- hn quantized to fp8 at scale 1 (|hn| ~< 6).
- weights shipped as fp8(16*w) (sigma ~0.7); biases bq,bk,bv shipped as 16*b.
- q' = 16q, k' = 16k in fp8 (|q'| ~< 90)  -> scores acc = 256*(q.k)
- E = exp(acc * C^-0.5/256 - 2) in fp8 (max ~123; softmax shift-invariant,
  normalization deferred so the constant e^-2 cancels in rowsum)
- V' = 16v fp8; O_acc = V'.T @ E = 16*(sum E v); OT = O_acc * 2^-6 fp8 (max ~95)
- P_acc = OT.T@(16wp) = 4*(sum E v)@wp; final = P_acc * 1/(4*rowsum) + (x+bp):
  the ones-column for the rowsum matmul is memset to 4.0 so the reciprocal
  directly yields 1/(4S).
Per-element fp8 rel err ~2%; attention branch is only ~9% of output norm,
so final rel err ~7e-3 (simulated) vs the 2e-2 gate.

Layout strategy per core (T=1024 tokens, C=512 channels per batch, B=4):
- x DMA'd twice: bf16 channel-major x^T tiles via DMA-transpose (feeds
  GroupNorm stats + apply), f32 token-major (residual).
- GroupNorm stats per channel via bn_stats over tokens, tiny group-combine
  matmuls, fused tensor_scalar apply producing fp8 hn^T.
- scores^T[tk,tq] = K^T.T @ Q^T per (tk tile, tq 512-chunk); exp on ScalarE
  (scale+bias folded), fp8 E^T; row sums via DVE partial accumulation +
  one float32r ones-column matmul; normalization deferred past the output
  projection and fused into the final residual pass (scalar_tensor_tensor).
"""

import os
import sys

sys.path.insert(0, "/opt/trn_rl_repo")

import numpy as np
import ml_dtypes

import concourse.bass as bass
import concourse.mybir as mybir
import concourse.tile as tile
from concourse import bacc
from concourse.bass_utils import run_bass_kernel_spmd

BF16 = mybir.dt.bfloat16
FP8 = mybir.dt.float8e4
F32 = mybir.dt.float32
AF = mybir.ActivationFunctionType
ALU = mybir.AluOpType
DR = mybir.MatmulPerfMode.DoubleRow

NCORES = 8
B = 4          # batches per core
T = 1024       # tokens (h*w) per batch
C = 512        # channels
G = 32         # groups
GS = C // G    # 16 channels per group
NC4 = C // 128   # 4 channel chunks
NT8 = T // 128   # 8 token tiles
EPS = 1e-6
SCALE = C ** -0.5
OFF = 2.0        # exp offset: E = exp(s - OFF), cancels via deferred norm
WS = 16.0        # weight/bias prescale (host side)
OSH = 0.015625   # 2^-6 OT writeout scale
RS = 4.0         # rowsum ones value: 256*OSH so rcols = 1/(4S) matches P_acc scale


def build_kernel(repeat=1, bench=False):
    nc = bacc.Bacc("TRN2", target_bir_lowering=False, debug=False)

    if bench:
        # timing-only variant: no external I/O beyond a tiny debug output, so
        # the axon tunnel ships ~nothing per call; x lives in internal DRAM
        # (zero-filled below), weights/biases are memset on SBUF directly.
        xt_bf = nc.dram_tensor("xt_bf_i", [B, T, C], BF16, kind="Internal")
        x_f = nc.dram_tensor("x_f_i", [B, T, C], F32, kind="Internal")
        out_d = nc.dram_tensor("out_i", [B, T, C], F32, kind="Internal")
        out_dbg = nc.dram_tensor("out_dbg", [1, T], F32, kind="ExternalOutput")
    else:
        xt_bf = nc.dram_tensor("xt_bf", [B, T, C], BF16, kind="ExternalInput")
        x_f = nc.dram_tensor("x_f", [B, T, C], F32, kind="ExternalInput")
        out_d = nc.dram_tensor("out", [B, T, C], F32, kind="ExternalOutput")
    if not bench:
        w_d = {}
        for w in ("wq", "wk", "wv", "wp"):
            w_d[w] = nc.dram_tensor(w, [C, C], FP8, kind="ExternalInput")
        b_d = {}
        for b in ("bq", "bk", "gns", "gnb"):
            b_d[b] = nc.dram_tensor(b, [C], F32, kind="ExternalInput")
        for b in ("bv", "bp"):
            b_d[b] = nc.dram_tensor(b, [C], F32, kind="ExternalInput")
        gsel_d = nc.dram_tensor("gsel", [C, G], F32, kind="ExternalInput")
        gselT_d = nc.dram_tensor("gselT", [G, C], F32, kind="ExternalInput")

    with tile.TileContext(nc) as tc:
        with tc.tile_pool(name="const", bufs=1) as const, \
             tc.tile_pool(name="work", bufs=1) as work, \
             tc.tile_pool(name="psum", bufs=6, space="PSUM") as psum, \
             tc.tile_pool(name="dscratch", bufs=2, space="DRAM") as dscratch:

            # ---- constants ----
            w_t = {}
            for w in ("wq", "wk", "wv", "wp"):
                wt = const.tile([128, NC4, C], FP8, name=f"{w}_t")
                if bench:
                    nc.vector.memset(wt, 0.0)
                else:
                    nc.sync.dma_start(out=wt, in_=w_d[w].ap().rearrange("(i p) c -> p i c", p=128))
                w_t[w] = wt
            b_c = {}
            for b in ("bq", "bk", "gns", "gnb"):
                bc = const.tile([128, NC4], F32, name=f"{b}_c")
                if bench:
                    nc.vector.memset(bc, 0.0)
                else:
                    nc.sync.dma_start(out=bc, in_=b_d[b].ap().rearrange("(i p) -> p i", p=128))
                b_c[b] = bc
            def row_bcast(dram_t):
                src = dram_t.ap()
                return bass.AP(tensor=src.tensor, offset=src.offset, ap=[[0, 128]] + list(src.ap))

            bv_b = const.tile([128, C], F32)
            bp_b = const.tile([128, C], F32)
            gsel_t = const.tile([128, NC4, G], F32)
            gselT_t = const.tile([G, C], F32)
            if bench:
                nc.vector.memset(bv_b, 0.0)
                nc.vector.memset(bp_b, 0.0)
                nc.vector.memset(gsel_t, 1.0 / GS)
                nc.vector.memset(gselT_t, 1.0)
            else:
                nc.sync.dma_start(out=bv_b, in_=row_bcast(b_d["bv"]))
                nc.sync.dma_start(out=bp_b, in_=row_bcast(b_d["bp"]))
                nc.sync.dma_start(out=gsel_t, in_=gsel_d.ap().rearrange("(i p) g -> p i g", p=128))
                nc.sync.dma_start(out=gselT_t, in_=gselT_d.ap())
            ones_f = const.tile([128, 1], F32)
            nc.vector.memset(ones_f, RS)
            ones_r = const.tile([128, 1], mybir.dt.float32r)
            nc.vector.tensor_copy(out=ones_r, in_=ones_f)
            eps32 = const.tile([G, 1], F32)
            nc.vector.memset(eps32, EPS)
            noff = const.tile([128, 1], F32)
            nc.vector.memset(noff, -OFF)
            # prime the ScalarE exp/ln table set while the weight DMAs run, so
            # the first GroupNorm rstd doesn't eat the ~2.7us table load
            warm = const.tile([1, 1], F32)
            nc.scalar.activation(out=warm, in_=eps32[0:1, 0:1], func=AF.Exp, scale=1.0)

            if bench:
                # zero-fill internal x so the repeated body is NaN-free
                zf = work.tile([128, NT8, C], F32, tag="xf", bufs=2)
                nc.vector.memset(zf, 0.0)
                zb = work.tile([128, NT8, C], BF16, tag="zb")
                nc.vector.memset(zb, 0.0)
                for ib in range(B):
                    nc.sync.dma_start(out=x_f.ap()[ib].rearrange("(i p) c -> p i c", p=128), in_=zf)
                    nc.sync.dma_start(out=xt_bf.ap()[ib].rearrange("(i p) c -> p i c", p=128), in_=zb)

            # bench mode uses a hardware loop: keeps the NEFF small so the
            # marginal-repeat measurement isn't contaminated by per-call
            # NEFF-shipping costs that scale with instruction count
            import contextlib
            rep_ctx = tc.For_i(0, repeat) if (bench and repeat > 1) else contextlib.nullcontext()
            with rep_ctx:
              # ---- prologue: GroupNorm for all batches (keeps the steady-state
              # PE stream free of GN work at batch boundaries) ----
              hnT_l = []
              for ib in range(B):
                  xT = work.tile([128, NC4, T], BF16, tag="xT", bufs=2, name=f"xT{ib}")
                  for ci in range(NC4):
                      nc.sync.dma_start(
                          out=xT[:, ci, :],
                          in_=xt_bf.ap()[ib, :, ci * 128:(ci + 1) * 128],
                          transpose=True,
                      )
                  bn6 = work.tile([128, NC4, 2, 6], F32, tag="bn6", bufs=2, name=f"bn6_{ib}")
                  mv = work.tile([128, NC4, 2], F32, tag="mv", bufs=2, name=f"mv{ib}")
                  st2 = work.tile([128, NC4, 2], F32, tag="st2", bufs=2, name=f"st2_{ib}")
                  for ci in range(NC4):
                      nc.vector.bn_stats(out=bn6[:, ci, 0, :], in_=xT[:, ci, 0:512])
                      nc.vector.bn_stats(out=bn6[:, ci, 1, :], in_=xT[:, ci, 512:1024])
                      nc.vector.bn_aggr(out=mv[:, ci, :], in_=bn6[:, ci, :, :])
                      # st2 = [mean_c, var_c + mean_c^2]
                      nc.vector.tensor_tensor(out=st2[:, ci, 1:2], in0=mv[:, ci, 0:1], in1=mv[:, ci, 0:1], op=ALU.mult)
                      nc.vector.tensor_tensor(out=st2[:, ci, 1:2], in0=st2[:, ci, 1:2], in1=mv[:, ci, 1:2], op=ALU.add)
                      nc.vector.tensor_copy(out=st2[:, ci, 0:1], in_=mv[:, ci, 0:1])
                  gst = psum.tile([G, 2], F32, tag="small", bufs=2, name=f"gst{ib}")
                  for ci in range(NC4):
                      nc.tensor.matmul(gst, gsel_t[:, ci, :], st2[:, ci, :],
                                       start=(ci == 0), stop=(ci == NC4 - 1))
                  # gq cols: 0=mean_g 1=Ex2_g 2=mean^2 3=var 4=ln(var+eps) 5=rstd 6=mean
                  gq = work.tile([G, 7], F32, tag="gq", bufs=2, name=f"gq{ib}")
                  nc.vector.tensor_copy(out=gq[:, 0:2], in_=gst)
                  nc.vector.tensor_tensor(out=gq[:, 2:3], in0=gq[:, 0:1], in1=gq[:, 0:1], op=ALU.mult)
                  nc.vector.tensor_tensor(out=gq[:, 3:4], in0=gq[:, 1:2], in1=gq[:, 2:3], op=ALU.subtract)
                  nc.scalar.activation(out=gq[:, 4:5], in_=gq[:, 3:4], func=AF.Ln, bias=eps32, scale=1.0)
                  nc.scalar.activation(out=gq[:, 5:6], in_=gq[:, 4:5], func=AF.Exp, scale=-0.5)
                  nc.vector.tensor_copy(out=gq[:, 6:7], in_=gq[:, 0:1])
                  # expand to channels + per-channel affine [r', m']
                  rm = work.tile([128, NC4, 2], F32, tag="rm", bufs=2, name=f"rm{ib}")
                  for ci in range(NC4):
                      chq = psum.tile([128, 2], F32, tag="small", bufs=2, name=f"chq{ib}_{ci}")
                      nc.tensor.matmul(chq, gselT_t[:, ci * 128:(ci + 1) * 128], gq[:, 5:7],
                                       start=True, stop=True)
                      # r' = rstd * gns ; m' = gnb - mean * r'
                      nc.vector.tensor_tensor(out=rm[:, ci, 0:1], in0=chq[:, 0:1], in1=b_c["gns"][:, ci:ci + 1], op=ALU.mult)
                      nc.vector.tensor_tensor(out=rm[:, ci, 1:2], in0=chq[:, 1:2], in1=rm[:, ci, 0:1], op=ALU.mult)
                      nc.vector.tensor_tensor(out=rm[:, ci, 1:2], in0=b_c["gnb"][:, ci:ci + 1], in1=rm[:, ci, 1:2], op=ALU.subtract)
                  hnT = work.tile([128, NC4, T], FP8, tag="hnT", bufs=B, name=f"hnT{ib}")
                  for ci in range(NC4):
                      nc.vector.tensor_scalar(
                          out=hnT[:, ci, :], in0=xT[:, ci, :],
                          scalar1=rm[:, ci, 0:1], scalar2=rm[:, ci, 1:2],
                          op0=ALU.mult, op1=ALU.add)
                  hnT_l.append(hnT)

              for ib in range(B):
                  hnT = hnT_l[ib]
                  xf = work.tile([128, NT8, C], F32, tag="xf", bufs=2, name=f"xf{ib}")
                  nc.sync.dma_start(out=xf, in_=x_f.ap()[ib].rearrange("(i p) c -> p i c", p=128))

                  # ---- QKV (DoubleRow: 2 accumulation steps over K=512) ----
                  qT = work.tile([128, NC4, T], FP8, tag="qT", bufs=1, name=f"qT{ib}")
                  kT = work.tile([128, NC4, T], FP8, tag="kT", bufs=1, name=f"kT{ib}")
                  for (wname, bname, dst) in (("wq", "bq", qT), ("wk", "bk", kT)):
                      for co in range(NC4):
                          for h in range(2):
                              acc = psum.tile([128, 512], F32, tag="mm", name=f"acc_{wname}{ib}_{co}_{h}")
                              for kp in range(2):
                                  nc.tensor.matmul(
                                      acc, w_t[wname][:, 2 * kp:2 * kp + 2, co * 128:(co + 1) * 128],
                                      hnT[:, 2 * kp:2 * kp + 2, h * 512:(h + 1) * 512],
                                      start=(kp == 0), stop=(kp == 1), perf_mode=DR)
                              nc.vector.tensor_scalar(
                                  out=dst[:, co, h * 512:(h + 1) * 512], in0=acc,
                                  scalar1=b_c[bname][:, co:co + 1], scalar2=None, op0=ALU.add)
                  V = work.tile([128, NT8, C], FP8, tag="V", bufs=1, name=f"V{ib}")
                  for it in range(NT8):
                      acc = psum.tile([128, 512], F32, tag="mm", name=f"acc_v{ib}_{it}")
                      for kp in range(2):
                          nc.tensor.matmul(acc, hnT[:, 2 * kp:2 * kp + 2, it * 128:(it + 1) * 128],
                                           w_t["wv"][:, 2 * kp:2 * kp + 2, :],
                                           start=(kp == 0), stop=(kp == 1), perf_mode=DR)
                      nc.vector.tensor_tensor(out=V[:, it, :], in0=acc, in1=bv_b, op=ALU.add)

                  # ---- scores^T -> exp -> E, and row sums ----
                  E = work.tile([128, NT8, T], FP8, tag="E", bufs=1, name=f"E{ib}")
                  srow = work.tile([1, T], F32, tag="srow", bufs=2, name=f"srow{ib}")
                  ssb_l = []
                  for h in range(2):
                      ssb = work.tile([128, 512], mybir.dt.float32r, tag="ssb", bufs=2, name=f"ssb{ib}_{h}")
                      for tk in range(NT8):
                          acc = psum.tile([128, 512], F32, tag="mm", name=f"acc_s{ib}_{h}_{tk}")
                          for kp in range(2):
                              nc.tensor.matmul(acc, kT[:, 2 * kp:2 * kp + 2, tk * 128:(tk + 1) * 128],
                                               qT[:, 2 * kp:2 * kp + 2, h * 512:(h + 1) * 512],
                                               start=(kp == 0), stop=(kp == 1), perf_mode=DR)
                          nc.scalar.activation(out=E[:, tk, h * 512:(h + 1) * 512], in_=acc,
                                               func=AF.Exp, scale=SCALE / (WS * WS), bias=noff)
                          if tk == 0:
                              nc.vector.tensor_copy(out=ssb, in_=E[:, tk, h * 512:(h + 1) * 512])
                          else:
                              nc.vector.tensor_tensor(out=ssb, in0=ssb, in1=E[:, tk, h * 512:(h + 1) * 512], op=ALU.add)
                      ssb_l.append(ssb)

                  # ---- O^T = V.T @ E^T (DoubleRow over token pairs) ----
                  # row sums are emitted after the FIRST O group: by then the serial
                  # DVE partial-sum chain has drained (no PE wait), and the rcols
                  # DRAM bounce completes mid-O-phase, well before the projection
                  OT = work.tile([128, NC4, T], FP8, tag="OT", bufs=1, name=f"OT{ib}")
                  for co in range(NC4):
                      for h in range(2):
                          acc = psum.tile([128, 512], F32, tag="mm", name=f"acc_o{ib}_{co}_{h}")
                          for tp in range(4):
                              nc.tensor.matmul(acc, V[:, 2 * tp:2 * tp + 2, co * 128:(co + 1) * 128],
                                               E[:, 2 * tp:2 * tp + 2, h * 512:(h + 1) * 512],
                                               start=(tp == 0), stop=(tp == 3), perf_mode=DR)
                          nc.scalar.activation(out=OT[:, co, h * 512:(h + 1) * 512], in_=acc,
                                               func=AF.Copy, scale=OSH)
                          if co == 0 and h == 0:
                              for hs in range(2):
                                  sums = psum.tile([1, 512], F32, tag="small", bufs=2, name=f"sums{ib}_{hs}")
                                  # float32r runs at full PE rate for N>=256 (plain fp32 is 4x slower)
                                  nc.tensor.matmul(sums, ones_r, ssb_l[hs], start=True, stop=True)
                                  nc.vector.reciprocal(out=srow[:, hs * 512:(hs + 1) * 512], in_=sums)
                              # bounce recip row -> per-token columns
                              dsc = dscratch.tile([T], F32, tag="dsc", name=f"dsc{ib}")
                              nc.sync.dma_start(out=dsc, in_=srow[0:1, :])
                              rcols = work.tile([128, NT8], F32, tag="rcols", bufs=2, name=f"rcols{ib}")
                              nc.sync.dma_start(out=rcols, in_=dsc.rearrange("(j p) -> p j", p=128))

                  # ---- proj + normalize + residual ----
                  xpb = work.tile([128, NT8, C], F32, tag="xpb", bufs=1, name=f"xpb{ib}")
                  for it in range(NT8):
                      nc.gpsimd.tensor_tensor(out=xpb[:, it, :], in0=xf[:, it, :], in1=bp_b, op=ALU.add)
                  fin = work.tile([128, NT8, C], F32, tag="fin", bufs=1, name=f"fin{ib}")
                  for it in range(NT8):
                      acc = psum.tile([128, 512], F32, tag="mm", name=f"acc_p{ib}_{it}")
                      for kp in range(2):
                          nc.tensor.matmul(acc, OT[:, 2 * kp:2 * kp + 2, it * 128:(it + 1) * 128],
                                           w_t["wp"][:, 2 * kp:2 * kp + 2, :],
                                           start=(kp == 0), stop=(kp == 1), perf_mode=DR)
                      nc.vector.scalar_tensor_tensor(out=fin[:, it, :], in0=acc,
                                                     scalar=rcols[:, it:it + 1], in1=xpb[:, it, :],
                                                     op0=ALU.mult, op1=ALU.add)
                  nc.sync.dma_start(out=out_d.ap()[ib].rearrange("(i p) c -> p i c", p=128), in_=fin)
            if bench:
                nc.sync.dma_start(out=out_dbg.ap(), in_=srow)

    nc.compile()
    return nc


def make_selectors():
    cc = np.arange(C)
    gg = np.arange(G)
    sel = (cc[:, None] // GS == gg[None, :]).astype(np.float32)
    gsel = sel / GS            # [C, G] averaging
    gselT = sel.T.copy()       # [G, C] expand
    return gsel, gselT


_NC_CACHE = {}


def _get_nc(repeat=1, bench=False):
    key = (repeat, bench)
    if key not in _NC_CACHE:
        _NC_CACHE[key] = build_kernel(repeat, bench)
    return _NC_CACHE[key]


def make_in_maps(x, norm_scale, norm_bias, wq, bq, wk, bk, wv, bv, wp, bp):
    x = np.asarray(x, dtype=np.float32)
    b, h, w, c = x.shape
    assert (b, h * w, c) == (B * NCORES, T, C)
    xr = np.ascontiguousarray(x.reshape(b, h * w, c))
    xr_bf = xr.astype(ml_dtypes.bfloat16)
    gsel, gselT = make_selectors()
    common = {
        "wq": (WS * np.asarray(wq, np.float32)).astype(ml_dtypes.float8_e4m3),
        "wk": (WS * np.asarray(wk, np.float32)).astype(ml_dtypes.float8_e4m3),
        "wv": (WS * np.asarray(wv, np.float32)).astype(ml_dtypes.float8_e4m3),
        "wp": (WS * np.asarray(wp, np.float32)).astype(ml_dtypes.float8_e4m3),
        "bq": WS * np.asarray(bq, np.float32), "bk": WS * np.asarray(bk, np.float32),
        "bv": WS * np.asarray(bv, np.float32), "bp": np.asarray(bp, np.float32),
        "gns": np.asarray(norm_scale, np.float32), "gnb": np.asarray(norm_bias, np.float32),
        "gsel": gsel, "gselT": gselT,
    }
    in_maps = []
    for i in range(NCORES):
        sl = slice(i * B, (i + 1) * B)
        in_maps.append({"xt_bf": xr_bf[sl], "x_f": xr[sl], **common})
    return in_maps


def run(in_maps, **kw):
    nc = _get_nc()
    try:
        res = run_bass_kernel_spmd(nc, in_maps, core_ids=list(range(NCORES)), **kw)
    except Exception:
        # transient NRT device wedges happen; one retry is usually enough
        import time as _time
        _time.sleep(2.0)
        res = run_bass_kernel_spmd(nc, in_maps, core_ids=list(range(NCORES)), **kw)
    outs = [r["out"] for r in res.results]
    full = np.concatenate(outs, axis=0).reshape(B * NCORES, 32, 32, C)
    return full, res


def kernel(x, norm_scale, norm_bias, wq, bq, wk, bk, wv, bv, wp, bp):
    in_maps = make_in_maps(x, norm_scale, norm_bias, wq, bq, wk, bk, wv, bv, wp, bp)
    full, _ = run(in_maps)
    return full


if __name__ == "__main__":
    rng = np.random.default_rng(0)
    inputs = {
        "x": rng.standard_normal((32, 32, 32, 512), dtype=np.float32),
        "norm_scale": np.ones(512, np.float32),
        "norm_bias": np.zeros(512, np.float32),
    }
    s = 1.0 / np.sqrt(512)
    for nm in ("q", "k", "v", "p"):
        inputs[f"w{nm}"] = rng.standard_normal((512, 512), dtype=np.float32) * s
        inputs[f"b{nm}"] = np.zeros(512, np.float32)
    out = kernel(**inputs)
    print("out", out.shape, out.dtype, float(np.abs(out).max()))
